# revision 30
# baseline (speedup 1.0000x reference)
"""Trainium2 Bass kernel for AudioPreprocessingLayer.

Computes: floor(log2(mel_fb @ (rfft(x*hamming, norm=forward).real ** 2)))
for x of shape (4096, 32, 512), sharded batch-wise across 8 NeuronCores.

Key ideas:
  - rfft(.).real is a matmul with the cosine matrix C[n,k] = cos(2*pi*k*n/512)/512.
    The hamming window folds into it host-side: W = diag(hw) @ C.
  - Mel filterbank column 0 (DC bin) is structurally zero, so only bins 1..256
    are computed -> 256 = 2x128 clean chunks (checked at runtime, with a
    257-bin fallback).
  - x is converted once to bf16; the on-chip transpose runs as REGULAR bf16
    matmuls against an identity (1 cycle/row AND counts as PE activity, so
    the HAM clock gate stays at 2.4 GHz — transpose-mode matmuls don't).
  - floor(log2(m)) for positive fp32 m is exactly
    max(bitcast_int32(m) >> 23, 75) - 127   (the max() also maps the
    mels==0 -> eps=2^-52 case to -52 exactly).
  - Rows are mapped to partitions in blocks of 4 (row = 4p+j) so every DMA
    descriptor covers 4 consecutive DRAM rows (8 KB in, 320 B out).
"""

import os
import sys

for _p in ("/opt/trn_rl_repo",):
    if _p not in sys.path and os.path.isdir(_p):
        sys.path.append(_p)

import numpy as np
import ml_dtypes

import concourse.bass as bass
from concourse import bacc, mybir
from concourse.tile import TileContext
from concourse.bass_utils import run_bass_kernel_spmd
from concourse.masks import make_identity

N_CORES = 8
B, T, FRAME = 4096, 32, 512
R_PER_CORE = (B // N_CORES) * T  # 16384 rows of length 512 per core
N_MELS = 20

f32 = mybir.dt.float32
f32r = mybir.dt.float32r
bf16 = mybir.dt.bfloat16
i32 = mybir.dt.int32


def _ceil_div(a, b):
    return (a + b - 1) // b


def build_graph(R=R_PER_CORE, NF=256, group_r=512, w_dtype=f32r):
    """Build the SPMD Bass graph for one core's shard.

    x:   [R, 512]  f32   rows to transform
    w:   [4, 128, NF] f32  cosine*window matrix, chunked along n
    fbt: [NFC, 128, N_MELS] bf16  mel filterbank transposed+chunked along freq
    out: [R, N_MELS] f32
    """
    assert R % group_r == 0 and group_r % 128 == 0
    RT = group_r // 128          # row subtiles per group (block size k)
    NG = R // group_r            # number of groups
    NQ = FRAME // 128            # 4 n-chunks
    NFC = _ceil_div(NF, 128)     # freq chunks
    f_sizes = [min(128, NF - 128 * c) for c in range(NFC)]

    nc = bacc.Bacc(None, target_bir_lowering=False)
    x_d = nc.declare_dram_parameter("x", [R, FRAME], f32, isOutput=False)
    w_d = nc.declare_dram_parameter("w", [NQ, 128, NF], f32, isOutput=False)
    fbt_d = nc.declare_dram_parameter("fbt", [NFC, 128, N_MELS], bf16, isOutput=False)
    out_d = nc.declare_dram_parameter("out", [R, N_MELS], f32, isOutput=True)

    with TileContext(nc) as tc:
        with (
            tc.tile_pool(name="consts", bufs=1) as consts,
            tc.tile_pool(name="xb", bufs=3) as xb_pool,
            tc.tile_pool(name="xq", bufs=2) as xq_pool,
            tc.tile_pool(name="mag", bufs=2) as mag_pool,
            tc.tile_pool(name="fin", bufs=3) as fin_pool,
            tc.tile_pool(name="ps_xt", bufs=3, space="PSUM") as ps_xt_pool,
            tc.tile_pool(name="ps_y", bufs=2, space="PSUM") as ps_y_pool,
            tc.tile_pool(name="ps_m", bufs=1, space="PSUM") as ps_m_pool,
        ):
            # ---- constants ----
            ident = consts.tile([128, 128], bf16)
            make_identity(nc, ident)

            w_sb = consts.tile([128, NQ, NF], f32)
            nc.sync.dma_start(out=w_sb, in_=w_d.rearrange("q p f -> p q f"))
            # fp32r operands must be produced pre-rounded; one-time copy
            w_r = consts.tile([128, NQ, NF], w_dtype)
            nc.vector.tensor_copy(w_r, w_sb)

            fbt_sb = consts.tile([128, NFC, N_MELS], bf16)
            nc.sync.dma_start(out=fbt_sb, in_=fbt_d.rearrange("c p m -> p c m"))

            # compute groups per DMA macro-group; first ones small so the
            # pipeline fills quickly
            n_groups = R // group_r
            if n_groups >= 8:
                gpm_list = [1, 3] + [4] * ((n_groups - 4) // 4)
            else:
                gpm_list = [1] * n_groups
            assert sum(gpm_list) == n_groups

            # flat per-group schedule; software-pipelined: group g+1's
            # transposes are emitted BEFORE group g's matmul-1 so the
            # in-order PE queue never stalls waiting on the PSUM copies
            groups = []   # (macro, gg) per group
            macros = []   # per macro: dict(m0, GPM, JT)
            m0 = 0
            for mg, GPM in enumerate(gpm_list):
                macros.append({"m0": m0, "GPM": GPM, "JT": GPM * RT})
                for gg in range(GPM):
                    groups.append((mg, gg))
                m0 += GPM * group_r

            st = {}  # per-group transpose-stage outputs

            def stage_T(g):
                mg, gg = groups[g]
                mac = macros[mg]
                if gg == 0:
                    # load macro as a CASTING DMA (f32 dram -> bf16 sbuf);
                    # row m0 + JT*p + j -> partition p, slot j (up to 32 KB
                    # contiguous DRAM per partition = big descriptors)
                    JT = mac["JT"]
                    xb_sb = xb_pool.tile([128, JT, FRAME], bf16, name="xb_sb")
                    src = x_d[
                        mac["m0"] : mac["m0"] + JT * 128, :
                    ].rearrange("(p j) n -> p j n", j=JT)
                    if mg == 0:
                        # per-n-chunk loads so the very first transposes
                        # can start after ~1/4 of the data has landed
                        for q in range(NQ):
                            ns = slice(q * 128, (q + 1) * 128)
                            nc.gpsimd.dma_start(
                                out=xb_sb[:, :, ns], in_=src[:, :, ns]
                            )
                    else:
                        nc.gpsimd.dma_start(out=xb_sb, in_=src)
                    mac["xb"] = xb_sb
                    mac["e_sb"] = fin_pool.tile(
                        [128, JT * N_MELS], i32, tag="e_sb", name="e_sb"
                    )
                xb_sb = mac["xb"]
                # transpose via REGULAR bf16 matmuls (counts for HAM);
                # one single-bank PSUM slot per n-chunk
                xq_sb = []
                for q in range(NQ):
                    t = ps_xt_pool.tile(
                        [128, group_r], f32, name=f"xt{q}", tag="xt"
                    )
                    for j in range(RT):
                        nc.tensor.matmul(
                            t[:, j * 128 : (j + 1) * 128],
                            xb_sb[:, gg * RT + j, q * 128 : (q + 1) * 128],
                            ident,
                            start=True,
                            stop=True,
                        )
                    # copy PSUM -> SBUF as f32r (exact for bf16-valued x,
                    # keeps matmul 1 all-32-bit with full-precision W)
                    dst = xq_pool.tile(
                        [128, group_r], f32r, name=f"xq{q}", tag=f"xq{q}"
                    )
                    xq_sb.append(dst)
                    if q < 3:
                        nc.vector.tensor_copy(dst, t)
                    else:
                        nc.scalar.copy(dst, t)
                st[g] = xq_sb

            def stage_M1(g):
                # matmul 1: yT[f, r] += W[n, f].T @ xT[n, r]; then square
                xq_sb = st.pop(g)
                y_ps = ps_y_pool.tile([128, NFC, group_r], f32, name="y_ps")
                for c in range(NFC):
                    fs = f_sizes[c]
                    for q in range(NQ):
                        nc.tensor.matmul(
                            y_ps[:fs, c, :],
                            w_r[:, q, 128 * c : 128 * c + fs],
                            xq_sb[q],
                            start=(q == 0),
                            stop=(q == NQ - 1),
                        )
                # square: magT = yT*yT (fused, psum -> sbuf bf16)
                mag_sb = mag_pool.tile([128, NFC, group_r], bf16, name="mag_sb")
                nc.scalar.activation(
                    mag_sb, y_ps, mybir.ActivationFunctionType.Square
                )
                st[("mag", g)] = mag_sb

            def stage_M2(g):
                mg, gg = groups[g]
                mac = macros[mg]
                mag_sb = st.pop(("mag", g))
                # matmul 2: mels[r, m] += magT[f, r].T @ fbt[f, m]
                mels_ps = ps_m_pool.tile([128, RT * N_MELS], f32, name="mels_ps")
                for j in range(RT):
                    for c in range(NFC):
                        fs = f_sizes[c]
                        nc.tensor.matmul(
                            mels_ps[:, j * N_MELS : (j + 1) * N_MELS],
                            mag_sb[:fs, c, j * 128 : (j + 1) * 128],
                            fbt_sb[:fs, c, :],
                            start=(c == 0),
                            stop=(c == NFC - 1),
                        )
                # exponent bits out of PSUM (rest of finalize is batched)
                nc.vector.tensor_scalar(
                    mac["e_sb"][:, gg * RT * N_MELS : (gg + 1) * RT * N_MELS],
                    mels_ps.bitcast(i32),
                    23,
                    None,
                    mybir.AluOpType.logical_shift_right,
                )
                if gg == mac["GPM"] - 1:
                    # finalize: floor(log2(m)) = max(bits >> 23, 75) - 127
                    JT = mac["JT"]
                    e_sb = mac["e_sb"]
                    ef_sb = fin_pool.tile([128, JT * N_MELS], f32, tag="ef_sb", name="ef_sb")
                    nc.vector.tensor_copy(ef_sb, e_sb)
                    o_sb = fin_pool.tile([128, JT * N_MELS], f32, tag="o_sb", name="o_sb")
                    nc.vector.tensor_scalar(
                        o_sb,
                        ef_sb,
                        75.0,
                        127.0,
                        mybir.AluOpType.max,
                        mybir.AluOpType.subtract,
                    )
                    # store: one DMA per macro, JT rows per partition
                    nc.sync.dma_start(
                        out=out_d[
                            mac["m0"] : mac["m0"] + JT * 128, :
                        ].rearrange("(p j) m -> p (j m)", j=JT),
                        in_=o_sb,
                    )

            for g in range(len(groups)):
                stage_T(g)
                stage_M1(g)
                stage_M2(g)
    nc.compile()
    return nc


def _prep_weights(filter_banks, hw):
    """Host-side: cosine*window matrix and chunked transposed filterbank."""
    fb = np.asarray(filter_banks, dtype=np.float32)
    n_mels, n_bins = fb.shape  # (20, 257)
    assert n_mels == N_MELS and n_bins == FRAME // 2 + 1

    if np.all(fb[:, 0] == 0.0):
        k0 = 1  # DC bin unused by the filterbank (structurally true)
    else:
        k0 = 0
    NF = n_bins - k0

    n = np.arange(FRAME, dtype=np.float64)
    k = np.arange(k0, n_bins, dtype=np.float64)
    C = np.cos(2.0 * np.pi * np.outer(n, k) / FRAME) / FRAME
    W = (np.asarray(hw, dtype=np.float64)[:, None] * C).astype(np.float32)
    NQ = FRAME // 128
    w_chunks = np.ascontiguousarray(W.reshape(NQ, 128, NF))

    NFC = _ceil_div(NF, 128)
    fbt = np.zeros((NFC, 128, N_MELS), dtype=ml_dtypes.bfloat16)
    fbT = fb[:, k0:].T.astype(ml_dtypes.bfloat16)  # [NF, 20]
    for c in range(NFC):
        fs = min(128, NF - 128 * c)
        fbt[c, :fs, :] = fbT[128 * c : 128 * c + fs, :]
    return w_chunks, fbt, NF


_CACHE = {}


def _get_graph(R, NF, group_r):
    key = (R, NF, group_r)
    if key not in _CACHE:
        _CACHE[key] = build_graph(R, NF, group_r)
    return _CACHE[key]


def kernel(inputs, filter_banks, hw, _trace=False, _group_r=512):
    x = np.ascontiguousarray(np.asarray(inputs, dtype=np.float32))
    assert x.shape == (B, T, FRAME), x.shape
    w_chunks, fbt, NF = _prep_weights(filter_banks, hw)

    shards = x.reshape(N_CORES, B // N_CORES * T, FRAME)
    nc = _get_graph(R_PER_CORE, NF, _group_r)
    in_maps = [
        {"x": shards[i], "w": w_chunks, "fbt": fbt} for i in range(N_CORES)
    ]
    res = run_bass_kernel_spmd(
        nc, in_maps, core_ids=list(range(N_CORES)), trace=_trace
    )
    out = np.stack([res.results[i]["out"] for i in range(N_CORES)], axis=0)
    out = out.reshape(B, T, N_MELS, 1).astype(np.float32)
    if _trace:
        kernel._last_result = res
    return out


# revision 31
# speedup vs baseline: 1.0069x; 1.0069x over previous
"""Trainium2 Bass kernel for AudioPreprocessingLayer.

Computes: floor(log2(mel_fb @ (rfft(x*hamming, norm=forward).real ** 2)))
for x of shape (4096, 32, 512), sharded batch-wise across 8 NeuronCores.

Key ideas:
  - rfft(.).real is a matmul with the cosine matrix C[n,k] = cos(2*pi*k*n/512)/512.
    The hamming window folds into it host-side: W = diag(hw) @ C.
  - Mel filterbank column 0 (DC bin) is structurally zero, so only bins 1..256
    are computed -> 256 = 2x128 clean chunks (checked at runtime, with a
    257-bin fallback).
  - x is converted once to bf16; the on-chip transpose runs as REGULAR bf16
    matmuls against an identity (1 cycle/row AND counts as PE activity, so
    the HAM clock gate stays at 2.4 GHz — transpose-mode matmuls don't).
  - floor(log2(m)) for positive fp32 m is exactly
    max(bitcast_int32(m) >> 23, 75) - 127   (the max() also maps the
    mels==0 -> eps=2^-52 case to -52 exactly).
  - Rows are mapped to partitions in blocks of 4 (row = 4p+j) so every DMA
    descriptor covers 4 consecutive DRAM rows (8 KB in, 320 B out).
"""

import os
import sys

for _p in ("/opt/trn_rl_repo",):
    if _p not in sys.path and os.path.isdir(_p):
        sys.path.append(_p)

import numpy as np
import ml_dtypes

import concourse.bass as bass
from concourse import bacc, mybir
from concourse.tile import TileContext
from concourse.bass_utils import run_bass_kernel_spmd
from concourse.masks import make_identity

N_CORES = 8
B, T, FRAME = 4096, 32, 512
R_PER_CORE = (B // N_CORES) * T  # 16384 rows of length 512 per core
N_MELS = 20

f32 = mybir.dt.float32
f32r = mybir.dt.float32r
bf16 = mybir.dt.bfloat16
i32 = mybir.dt.int32


def _ceil_div(a, b):
    return (a + b - 1) // b


def build_graph(R=R_PER_CORE, NF=256, group_r=512, w_dtype=f32r):
    """Build the SPMD Bass graph for one core's shard.

    x:   [R, 512]  f32   rows to transform
    w:   [4, 128, NF] f32  cosine*window matrix, chunked along n
    fbt: [NFC, 128, N_MELS] bf16  mel filterbank transposed+chunked along freq
    out: [R, N_MELS] f32
    """
    assert R % group_r == 0 and group_r % 128 == 0
    RT = group_r // 128          # row subtiles per group (block size k)
    NG = R // group_r            # number of groups
    NQ = FRAME // 128            # 4 n-chunks
    NFC = _ceil_div(NF, 128)     # freq chunks
    f_sizes = [min(128, NF - 128 * c) for c in range(NFC)]

    nc = bacc.Bacc(None, target_bir_lowering=False)
    x_d = nc.declare_dram_parameter("x", [R, FRAME], f32, isOutput=False)
    w_d = nc.declare_dram_parameter("w", [NQ, 128, NF], f32, isOutput=False)
    fbt_d = nc.declare_dram_parameter("fbt", [NFC, 128, N_MELS], bf16, isOutput=False)
    out_d = nc.declare_dram_parameter("out", [R, N_MELS], f32, isOutput=True)

    with TileContext(nc) as tc:
        with (
            tc.tile_pool(name="consts", bufs=1) as consts,
            tc.tile_pool(name="xb", bufs=3) as xb_pool,
            tc.tile_pool(name="xq", bufs=2) as xq_pool,
            tc.tile_pool(name="mag", bufs=2) as mag_pool,
            tc.tile_pool(name="fin", bufs=3) as fin_pool,
            tc.tile_pool(name="ps_xt", bufs=3, space="PSUM") as ps_xt_pool,
            tc.tile_pool(name="ps_y", bufs=2, space="PSUM") as ps_y_pool,
            tc.tile_pool(name="ps_m", bufs=1, space="PSUM") as ps_m_pool,
        ):
            # ---- constants ----
            ident = consts.tile([128, 128], bf16)
            make_identity(nc, ident)

            w_sb = consts.tile([128, NQ, NF], f32)
            nc.sync.dma_start(out=w_sb, in_=w_d.rearrange("q p f -> p q f"))
            # fp32r operands must be produced pre-rounded; one-time copy
            w_r = consts.tile([128, NQ, NF], w_dtype)
            nc.vector.tensor_copy(w_r, w_sb)

            fbt_sb = consts.tile([128, NFC, N_MELS], bf16)
            nc.sync.dma_start(out=fbt_sb, in_=fbt_d.rearrange("c p m -> p c m"))

            # compute groups per DMA macro-group; first ones small so the
            # pipeline fills quickly
            n_groups = R // group_r
            if n_groups >= 8:
                gpm_list = [1, 3] + [4] * ((n_groups - 4) // 4)
            else:
                gpm_list = [1] * n_groups
            assert sum(gpm_list) == n_groups

            # flat per-group schedule; software-pipelined: group g+1's
            # transposes are emitted BEFORE group g's matmul-1 so the
            # in-order PE queue never stalls waiting on the PSUM copies
            groups = []   # (macro, gg) per group
            macros = []   # per macro: dict(m0, GPM, JT)
            m0 = 0
            for mg, GPM in enumerate(gpm_list):
                macros.append({"m0": m0, "GPM": GPM, "JT": GPM * RT})
                for gg in range(GPM):
                    groups.append((mg, gg))
                m0 += GPM * group_r

            st = {}  # per-group transpose-stage outputs

            def stage_T(g):
                mg, gg = groups[g]
                mac = macros[mg]
                if gg == 0:
                    # load macro as a CASTING DMA (f32 dram -> bf16 sbuf);
                    # row m0 + JT*p + j -> partition p, slot j (up to 32 KB
                    # contiguous DRAM per partition = big descriptors)
                    JT = mac["JT"]
                    xb_sb = xb_pool.tile([128, JT, FRAME], bf16, name="xb_sb")
                    nc.gpsimd.dma_start(
                        out=xb_sb,
                        in_=x_d[
                            mac["m0"] : mac["m0"] + JT * 128, :
                        ].rearrange("(p j) n -> p j n", j=JT),
                    )
                    mac["xb"] = xb_sb
                    mac["e_sb"] = fin_pool.tile(
                        [128, JT * N_MELS], i32, tag="e_sb", name="e_sb"
                    )
                xb_sb = mac["xb"]
                # transpose via REGULAR bf16 matmuls (counts for HAM);
                # one single-bank PSUM slot per n-chunk
                xq_sb = []
                for q in range(NQ):
                    t = ps_xt_pool.tile(
                        [128, group_r], f32, name=f"xt{q}", tag="xt"
                    )
                    for j in range(RT):
                        nc.tensor.matmul(
                            t[:, j * 128 : (j + 1) * 128],
                            xb_sb[:, gg * RT + j, q * 128 : (q + 1) * 128],
                            ident,
                            start=True,
                            stop=True,
                        )
                    # copy PSUM -> SBUF as f32r (exact for bf16-valued x,
                    # keeps matmul 1 all-32-bit with full-precision W)
                    dst = xq_pool.tile(
                        [128, group_r], f32r, name=f"xq{q}", tag=f"xq{q}"
                    )
                    xq_sb.append(dst)
                    if q % 2 == 0:
                        nc.vector.tensor_copy(dst, t)
                    else:
                        nc.scalar.copy(dst, t)
                st[g] = xq_sb

            def stage_M1(g):
                # matmul 1: yT[f, r] += W[n, f].T @ xT[n, r]; then square
                xq_sb = st.pop(g)
                y_ps = ps_y_pool.tile([128, NFC, group_r], f32, name="y_ps")
                for c in range(NFC):
                    fs = f_sizes[c]
                    for q in range(NQ):
                        nc.tensor.matmul(
                            y_ps[:fs, c, :],
                            w_r[:, q, 128 * c : 128 * c + fs],
                            xq_sb[q],
                            start=(q == 0),
                            stop=(q == NQ - 1),
                        )
                # square: magT = yT*yT (fused, psum -> sbuf bf16)
                mag_sb = mag_pool.tile([128, NFC, group_r], bf16, name="mag_sb")
                nc.scalar.activation(
                    mag_sb, y_ps, mybir.ActivationFunctionType.Square
                )
                st[("mag", g)] = mag_sb

            def stage_M2(g):
                mg, gg = groups[g]
                mac = macros[mg]
                mag_sb = st.pop(("mag", g))
                # matmul 2: mels[r, m] += magT[f, r].T @ fbt[f, m]
                mels_ps = ps_m_pool.tile([128, RT * N_MELS], f32, name="mels_ps")
                for j in range(RT):
                    for c in range(NFC):
                        fs = f_sizes[c]
                        nc.tensor.matmul(
                            mels_ps[:, j * N_MELS : (j + 1) * N_MELS],
                            mag_sb[:fs, c, j * 128 : (j + 1) * 128],
                            fbt_sb[:fs, c, :],
                            start=(c == 0),
                            stop=(c == NFC - 1),
                        )
                # exponent bits out of PSUM (rest of finalize is batched)
                nc.vector.tensor_scalar(
                    mac["e_sb"][:, gg * RT * N_MELS : (gg + 1) * RT * N_MELS],
                    mels_ps.bitcast(i32),
                    23,
                    None,
                    mybir.AluOpType.logical_shift_right,
                )
                if gg == mac["GPM"] - 1:
                    # finalize: floor(log2(m)) = max(bits >> 23, 75) - 127
                    JT = mac["JT"]
                    e_sb = mac["e_sb"]
                    ef_sb = fin_pool.tile([128, JT * N_MELS], f32, tag="ef_sb", name="ef_sb")
                    nc.vector.tensor_copy(ef_sb, e_sb)
                    o_sb = fin_pool.tile([128, JT * N_MELS], f32, tag="o_sb", name="o_sb")
                    nc.vector.tensor_scalar(
                        o_sb,
                        ef_sb,
                        75.0,
                        127.0,
                        mybir.AluOpType.max,
                        mybir.AluOpType.subtract,
                    )
                    # store: one DMA per macro, JT rows per partition
                    nc.sync.dma_start(
                        out=out_d[
                            mac["m0"] : mac["m0"] + JT * 128, :
                        ].rearrange("(p j) m -> p (j m)", j=JT),
                        in_=o_sb,
                    )

            for g in range(len(groups)):
                stage_T(g)
                stage_M1(g)
                stage_M2(g)
    nc.compile()
    return nc


def _prep_weights(filter_banks, hw):
    """Host-side: cosine*window matrix and chunked transposed filterbank."""
    fb = np.asarray(filter_banks, dtype=np.float32)
    n_mels, n_bins = fb.shape  # (20, 257)
    assert n_mels == N_MELS and n_bins == FRAME // 2 + 1

    if np.all(fb[:, 0] == 0.0):
        k0 = 1  # DC bin unused by the filterbank (structurally true)
    else:
        k0 = 0
    NF = n_bins - k0

    n = np.arange(FRAME, dtype=np.float64)
    k = np.arange(k0, n_bins, dtype=np.float64)
    C = np.cos(2.0 * np.pi * np.outer(n, k) / FRAME) / FRAME
    W = (np.asarray(hw, dtype=np.float64)[:, None] * C).astype(np.float32)
    NQ = FRAME // 128
    w_chunks = np.ascontiguousarray(W.reshape(NQ, 128, NF))

    NFC = _ceil_div(NF, 128)
    fbt = np.zeros((NFC, 128, N_MELS), dtype=ml_dtypes.bfloat16)
    fbT = fb[:, k0:].T.astype(ml_dtypes.bfloat16)  # [NF, 20]
    for c in range(NFC):
        fs = min(128, NF - 128 * c)
        fbt[c, :fs, :] = fbT[128 * c : 128 * c + fs, :]
    return w_chunks, fbt, NF


_CACHE = {}


def _get_graph(R, NF, group_r):
    key = (R, NF, group_r)
    if key not in _CACHE:
        _CACHE[key] = build_graph(R, NF, group_r)
    return _CACHE[key]


def kernel(inputs, filter_banks, hw, _trace=False, _group_r=512):
    x = np.ascontiguousarray(np.asarray(inputs, dtype=np.float32))
    assert x.shape == (B, T, FRAME), x.shape
    w_chunks, fbt, NF = _prep_weights(filter_banks, hw)

    shards = x.reshape(N_CORES, B // N_CORES * T, FRAME)
    nc = _get_graph(R_PER_CORE, NF, _group_r)
    in_maps = [
        {"x": shards[i], "w": w_chunks, "fbt": fbt} for i in range(N_CORES)
    ]
    res = run_bass_kernel_spmd(
        nc, in_maps, core_ids=list(range(N_CORES)), trace=_trace
    )
    out = np.stack([res.results[i]["out"] for i in range(N_CORES)], axis=0)
    out = out.reshape(B, T, N_MELS, 1).astype(np.float32)
    if _trace:
        kernel._last_result = res
    return out
